# revision 1
# baseline (speedup 1.0000x reference)
"""Multi-head attention (B=2, S=2048, EMB=1024, 16 heads) on 8 Trainium2 cores.

Sharding: core c -> batch c//4, head-group c%4 (4 heads = 256 projection dims).
Each core computes Q/K projections in transposed layout (head-dim on
partitions), V natural, attention without max subtraction (scores ~ N(0,1) in
fp32), the softmax denominator via a ones-column appended to V (free inside
the ctx matmul M=65), and a row-parallel partial of the output projection.
The host sums the 4 partials per batch and adds the output bias.

HW-measured matmul design rules (see microbenches):
  - back-to-back matmuls on the SAME PE row group cost ~449ns (LDWEIGHTS
    serializes); alternating row groups or full-K with bank alternation gets
    200-255ns (LDW hides behind the other stream).
  - accumulation chains into the same PSUM bank back-to-back cost ~365ns;
    interleaving two chains on different banks avoids the hazard.
So every matmul stream below interleaves two chains (alternating PSUM banks)
and, for K=64 score matmuls, alternates row groups via the hi (head) index.

The exp runs as [128, 1024] activations over 2-bank PSUM score tiles to
amortize the ~143ns per-instruction ACT overhead (ACT is ~128us total and
near-critical).
"""

import numpy as np

import concourse.tile as tile
from concourse import bacc, mybir
from concourse import bass_utils

EMB = 1024
S = 2048
B = 2
HPC = 4            # heads per core
DQ = HPC * 64      # 256 projection dims per core
NCORES = 8

F32 = mybir.dt.float32
F32R = mybir.dt.float32r
EXP = mybir.ActivationFunctionType.Exp

KT_E = EMB // 128  # 8 contraction tiles over EMB
NQC = S // 512     # 4 query chunks
NST = S // 128     # 16 sequence tiles

_NC = None
TRACE = False
LAST_RESULT = None
STAGE = "full"     # "proj"|"sc"|"ctx"|"norm"|"out"|"fullnodma"|"full"


def _mha(ctx, tc, xqT, xkT, xvT, wqT, wkT, wvT, woT, bq, bk, bv, out, bench_iters=None):
    nc = tc.nc

    cstp = ctx.enter_context(tc.tile_pool(name="const", bufs=1))
    xpool = ctx.enter_context(tc.tile_pool(name="xin", bufs=16))
    epool = ctx.enter_context(tc.tile_pool(name="exp", bufs=6))
    bpool = ctx.enter_context(tc.tile_pool(name="bcsb", bufs=2))
    opool = ctx.enter_context(tc.tile_pool(name="osb", bufs=3))
    upool = ctx.enter_context(tc.tile_pool(name="unorm", bufs=8))
    # PSUM: spool 4x[128,512] (4 banks, shared ring: scores / projections /
    # out-proj / bcast; depth 4 lets PE run ahead of ACT and absorb the
    # interleaved finish work) + ctx_ps 4x[65,512] (4 banks: four half-chains,
    # constant row-group per chain, consecutive matmuls alternate row groups
    # and banks) = 8 banks exactly.
    spool = ctx.enter_context(tc.tile_pool(name="scps", bufs=4, space="PSUM"))
    ctx_ps = ctx.enter_context(tc.tile_pool(name="ctxps", bufs=4, space="PSUM"))
    mpool = spool

    # ---- persistent SBUF tensors ----
    ones_row = cstp.tile([1, 512], F32R)
    nc.vector.memset(ones_row[:].bitcast(F32), 1.0)
    sel64 = cstp.tile([65, 64], F32R)           # one-hot: row 64 -> all cols
    nc.vector.memset(sel64[:].bitcast(F32), 0.0)
    nc.vector.memset(sel64[64:65, :].bitcast(F32), 1.0)
    rdens = []
    for i in range(4):   # rotating: row 64 = 1/denom, rows 0..63 zero
        r = cstp.tile([65, 512], F32R, name=f"rden{i}")
        nc.vector.memset(r[0:64, :].bitcast(F32), 0.0)
        rdens.append(r)

    wq_sb = cstp.tile([128, KT_E * DQ], F32R)   # [128, 2048]: wq_sb[p, n*256+m] = WqT[n*128+p, m]
    wk_sb = cstp.tile([128, KT_E * DQ], F32R)
    wv_sb = cstp.tile([128, KT_E * DQ], F32R)
    for sb, src in ((wq_sb, wqT), (wk_sb, wkT), (wv_sb, wvT)):
        nc.sync.dma_start(
            sb[:].rearrange("p (n m) -> p n m", n=KT_E),
            src.rearrange("(n p) m -> p n m", p=128),
        )
    wo_sb = cstp.tile([128, 2 * EMB], F32R)     # wo_sb[p, n*1024+f] = WoT[n*128+p, f]
    nc.sync.dma_start(
        wo_sb[:].rearrange("p (n m) -> p n m", n=2),
        woT.rearrange("(n p) m -> p n m", p=128),
    )
    # per-partition bias columns: b?c[p, dq] = bias[dq*128+p]
    bqc = cstp.tile([128, 2], F32)
    bkc = cstp.tile([128, 2], F32)
    for sb, src in ((bqc, bq), (bkc, bk)):
        nc.sync.dma_start(sb[:], src.rearrange("o (d p) -> p (o d)", p=128))
    bv_sb = cstp.tile([1, DQ], F32R)
    nc.sync.dma_start(bv_sb[:], bv[:])

    # results of phase 1 kept resident
    kT_sb = cstp.tile([128, 2 * S], F32R)       # [dq-block 2][s 2048]
    qT_sb = cstp.tile([128, 2 * S], F32R)
    ctxT_sb = cstp.tile([128, 2 * S], F32R)
    if STAGE == "out":
        nc.vector.memset(ctxT_sb[:].bitcast(F32), 0.001)
    v_sb = cstp.tile([128, NST * (HPC * 65)], F32R)  # per s-tile: 4 heads x (64 V + ones col)
    nc.vector.memset(
        v_sb[:].bitcast(F32).rearrange("p (t h m) -> p t h m", t=NST, h=HPC)[:, :, :, 64:65],
        1.0,
    )

    def body():
        _body(tc, nc, xqT, xkT, xvT, out, ones_row, sel64, rdens, wq_sb, wk_sb,
              wv_sb, wo_sb, bqc, bkc, bv_sb, kT_sb, qT_sb, ctxT_sb, v_sb,
              xpool, epool, bpool, opool, upool, mpool, spool, ctx_ps)

    if bench_iters:
        hints = (
            mybir.EngineType.PE,
            mybir.EngineType.Activation,
            mybir.EngineType.DVE,
            mybir.EngineType.SP,
            mybir.EngineType.Pool,
        )
        with tc.For_i(0, bench_iters, 1, hint_engines=hints):
            body()
    else:
        body()


def _body(tc, nc, xqT, xkT, xvT, out, ones_row, sel64, rdens, wq_sb, wk_sb,
          wv_sb, wo_sb, bqc, bkc, bv_sb, kT_sb, qT_sb, ctxT_sb, v_sb,
          xpool, epool, bpool, opool, upool, mpool, spool, ctx_ps):
    pending_norm = []
    fin_slices = []

    def _queue_finish(qc_done):
        fin_slices.extend(_finish_slices(nc, qc_done, pending_norm, rdens, sel64,
                                         opool, mpool, ctxT_sb, wo_sb, out))

    def _pop_finish():
        if fin_slices:
            fin_slices.pop(0)()

    def proj_dma(qc, pj, xsrc):
        xs = []
        for kt in range(KT_E):
            t = xpool.tile([128, 512], F32R, tag="xchunk", name=f"x_{qc}_{pj}_{kt}")
            nc.sync.dma_start(t[:], xsrc[kt, qc])
            xs.append(t)
        return xs

    def proj_qk(qc, pj, w_sb, xsrc, dst_sb, bcol, xs=None):
        if xs is None:
            xs = proj_dma(qc, pj, xsrc)
        # two dq chains interleaved -> PSUM banks alternate
        ps = [mpool.tile([128, 512], F32, tag="sc", name=f"p_{qc}_{pj}_{dq}")
              for dq in range(2)]
        for kt in range(KT_E):
            for dq in range(2):
                nc.tensor.matmul(
                    ps[dq][:], w_sb[:, kt * DQ + dq * 128: kt * DQ + dq * 128 + 128],
                    xs[kt][:], start=(kt == 0), stop=(kt == KT_E - 1),
                )
        for dq in range(2):
            nc.vector.tensor_scalar_add(
                dst_sb[:, dq * S + qc * 512: dq * S + qc * 512 + 512],
                ps[dq][:], bcol[:, dq: dq + 1],
            )

    # ---- phase 1: K and V for every chunk; Q only for chunk 0 (Q for
    # chunk qc+1 is emitted at the hp seam inside attention on chunk qc,
    # where the ACT-bound pipeline has PE slack and its DMA is long done).
    for qc in range(NQC):
        proj_qk(qc, 1, wk_sb, xkT, kT_sb, bkc)
        # V: natural layout, two chains at a time (padded [128,512] psum tiles
        # so the "mm" tag keeps a single slot size)
        xv = []
        for kt in range(KT_E):
            t = xpool.tile([128, 512], F32R, tag="xchunk", name=f"xv_{qc}_{kt}")
            nc.sync.dma_start(t[:], xvT[kt, qc])
            xv.append(t)
        if qc == 0:
            proj_qk(0, 0, wq_sb, xqT, qT_sb, bqc)
        for pair in range(2):
            vp = [mpool.tile([128, 512], F32, tag="sc", name=f"vps_{qc}_{pair}_{i}")
                  for i in range(2)]
            for kt in range(KT_E):
                for i in range(2):
                    sti = pair * 2 + i
                    nc.tensor.matmul(
                        vp[i][:, 0:DQ], xv[kt][:, sti * 128: sti * 128 + 128],
                        wv_sb[:, kt * DQ: kt * DQ + DQ],
                        start=(kt == 0), stop=False,
                    )
            for i in range(2):
                nc.tensor.matmul(
                    vp[i][:, 0:DQ], ones_row[0:1, 0:128], bv_sb[0:1, :],
                    start=False, stop=True,
                )
            for i in range(2):
                sti = pair * 2 + i
                st = qc * 4 + sti
                dst = v_sb[:, st * (HPC * 65): (st + 1) * (HPC * 65)]
                nc.vector.tensor_copy(
                    dst.rearrange("p (h m) -> p h m", h=HPC)[:, :, 0:64],
                    vp[i][:, 0:DQ].rearrange("p (h m) -> p h m", h=HPC),
                )

    # ---- phase 2: per query chunk: attention, then deferred norm/out-proj ----
    if STAGE == "proj":
        return
    xq_next = None
    for qc in range(NQC):
        if qc + 1 < NQC:
            # prefetch next chunk's Q input while this chunk's attention runs
            xq_next = proj_dma(qc + 1, 0, xqT)
        for hp in range(2):
            # four ctx half-chains per head-pair [hi][half]: constant
            # row-group per chain; consecutive matmuls alternate row groups
            # and PSUM banks (measured ~125ns/mm).
            cps = [
                [ctx_ps.tile([65, 512], F32, tag="ctx", name=f"ctx_{qc}_{hp}_{hi}_{half}")
                 for half in range(2)]
                for hi in range(2)
            ]

            def ctx_mms(es, kt):
                for hi in range(2):
                    h = hp * 2 + hi
                    vcol = kt * (HPC * 65) + h * 65
                    for half, b in enumerate((0, 64)):
                        nc.tensor.matmul(
                            cps[hi][half][:], v_sb[b:b + 64, vcol: vcol + 65],
                            es[hi][b:b + 64, :],
                            start=(kt == 0), stop=(kt == NST - 1),
                        )

            prev = None
            for kt in range(NST):
                es = []
                for hi in range(2):
                    base = 64 * hi
                    blk = hp * S
                    sc = spool.tile([128, 512], F32, tag="sc", name=f"sc_{qc}_{hp}_{kt}_{hi}")
                    nc.tensor.matmul(
                        sc[:],
                        kT_sb[base:base + 64, blk + kt * 128: blk + kt * 128 + 128],
                        qT_sb[base:base + 64, blk + qc * 512: blk + qc * 512 + 512],
                        start=True, stop=True,
                    )
                    e = epool.tile([128, 512], F32R, tag="e", name=f"e_{qc}_{hp}_{kt}_{hi}")
                    nc.scalar.activation(e[:], sc[:], EXP, scale=0.125)
                    es.append(e)
                if prev is not None and STAGE != "sc":
                    ctx_mms(*prev)
                prev = (es, kt)
                if kt % 2 == 1:
                    # interleave one slice of the previous chunk's deferred
                    # normalization / out-projection into the ACT-bound loop
                    _pop_finish()
            if STAGE != "sc":
                ctx_mms(*prev)
            for hi in range(2 if STAGE not in ("sc",) else 0):
                # combine the half-chains into SBUF; normalization +
                # out-projection are deferred one qc so they overlap the next
                # chunk's attention.
                tmpa = bpool.tile([65, 512], F32, tag="tmpa", name=f"tmpa_{qc}_{hp}_{hi}")
                nc.vector.tensor_copy(tmpa[:], cps[hi][0][:])
                u = upool.tile([65, 512], F32, tag="u", name=f"u_{qc}_{hp}_{hi}")
                nc.vector.tensor_add(u[:], cps[hi][1][:], tmpa[:])
                pending_norm.append((qc, hp, hi, u))
            if hp == 0 and qc + 1 < NQC:
                # Q projection for the next chunk, at the head-pair seam
                # (its input DMA was issued at the start of this chunk)
                proj_qk(qc + 1, 0, wq_sb, xqT, qT_sb, bqc, xs=xq_next)

        if STAGE in ("full", "norm", "out", "fullnodma"):
            _queue_finish(qc)
    while fin_slices:
        _pop_finish()


def _finish_slices(nc, qc, pending_norm, rdens, sel64, opool, mpool,
                   ctxT_sb, wo_sb, out):
    """Return the deferred norm + out-projection work for chunk qc as a list
    of closures, so the caller can interleave them into the next chunk's
    ACT-bound attention loop (PE/DVE fill the slack there)."""
    slices = []

    def recip_slice(hp, hi, u):
        def go():
            # dep-free DVE op: never head-of-line-blocks the DVE queue
            rden = rdens[2 * hp + hi]
            with nc.allow_low_precision(reason="f32r is 32-bit; rounding only"):
                nc.vector.reciprocal(rden[64:65, :], u[64:65, :])
        return go

    def bcast_slice(hp, hi, state):
        def go():
            # PE one-hot broadcast; its reciprocal ran >=4 pops earlier
            bps = mpool.tile([128, 512], F32, tag="sc", name=f"bc_{qc}_{hp}_{hi}")
            nc.tensor.matmul(bps[0:64, :], sel64[:], rdens[2 * hp + hi][:],
                             start=True, stop=True)
            state.append(bps)
        return go

    def mul_slice(hp, hi, u, state):
        def go():
            bps = state.pop(0)
            nc.vector.tensor_mul(
                ctxT_sb[64 * hi: 64 * hi + 64, hp * S + qc * 512: hp * S + qc * 512 + 512],
                u[0:64, :],
                bps[0:64, :],
            )
        return go

    def out_mm_slice(qt, state):
        def go():
            ps = [mpool.tile([128, 512], F32, tag="sc", name=f"ops_{qt}_{fc}")
                  for fc in range(2)]
            for dq in range(2):
                for fc in range(2):
                    nc.tensor.matmul(
                        ps[fc][:],
                        ctxT_sb[:, dq * S + qt * 128: dq * S + qt * 128 + 128],
                        wo_sb[:, dq * EMB + fc * 512: dq * EMB + fc * 512 + 512],
                        start=(dq == 0), stop=(dq == 1),
                    )
            state.append(ps)
        return go

    def out_drain_slice(qt, state):
        def go():
            ps = state.pop(0)
            ot = opool.tile([128, EMB], F32, tag="o", name=f"ot_{qt}")
            for fc in range(2):
                nc.vector.tensor_copy(ot[:, fc * 512: fc * 512 + 512], ps[fc][:])
            if STAGE != "fullnodma":
                nc.gpsimd.dma_start(out[qt * 128:(qt + 1) * 128, :], ot[:])
        return go

    mine = [p for p in pending_norm if p[0] == qc]
    if STAGE != "out":
        bstates = {}
        for pqc, hp, hi, u in mine:
            slices.append(recip_slice(hp, hi, u))
        for pqc, hp, hi, u in mine:
            bstates[(hp, hi)] = []
            slices.append(bcast_slice(hp, hi, bstates[(hp, hi)]))
        for pqc, hp, hi, u in mine:
            slices.append(mul_slice(hp, hi, u, bstates[(hp, hi)]))
    if STAGE != "norm":
        ostates = {}
        for qt4 in range(4):
            ostates[qt4] = []
            slices.append(out_mm_slice(qc * 4 + qt4, ostates[qt4]))
            if qt4 >= 1:
                slices.append(out_drain_slice(qc * 4 + qt4 - 1, ostates[qt4 - 1]))
        slices.append(out_drain_slice(qc * 4 + 3, ostates[3]))
    return slices


def _build_nc(bench_iters=None):
    from contextlib import ExitStack

    nc = bacc.Bacc("TRN2", target_bir_lowering=False, debug=False, num_devices=NCORES)
    xqT = nc.dram_tensor("xqT", [KT_E, NQC, 128, 512], F32R, kind="ExternalInput").ap()
    xkT = nc.dram_tensor("xkT", [KT_E, NQC, 128, 512], F32R, kind="ExternalInput").ap()
    xvT = nc.dram_tensor("xvT", [KT_E, NQC, 128, 512], F32R, kind="ExternalInput").ap()
    wqT = nc.dram_tensor("wqT", [EMB, DQ], F32R, kind="ExternalInput").ap()
    wkT = nc.dram_tensor("wkT", [EMB, DQ], F32R, kind="ExternalInput").ap()
    wvT = nc.dram_tensor("wvT", [EMB, DQ], F32R, kind="ExternalInput").ap()
    woT = nc.dram_tensor("woT", [DQ, EMB], F32R, kind="ExternalInput").ap()
    bq = nc.dram_tensor("bq", [1, DQ], F32, kind="ExternalInput").ap()
    bk = nc.dram_tensor("bk", [1, DQ], F32, kind="ExternalInput").ap()
    bv = nc.dram_tensor("bv", [1, DQ], F32R, kind="ExternalInput").ap()
    out = nc.dram_tensor("out", [S, EMB], F32, kind="ExternalOutput").ap()

    with ExitStack() as ctx:
        tc = ctx.enter_context(tile.TileContext(nc))
        _mha(ctx, tc, xqT, xkT, xvT, wqT, wkT, wvT, woT, bq, bk, bv, out,
             bench_iters=bench_iters)
    nc.compile()
    return nc


def _chunk_major(x):
    """[S, EMB] -> x.T chunked as [KT_E, NQC, 128, 512] (each chunk contiguous)."""
    xt = x.T  # [EMB, S]
    return np.ascontiguousarray(
        xt.reshape(KT_E, 128, NQC, 512).transpose(0, 2, 1, 3)
    )


def kernel(query, key, value, Wq, bq, Wk, bk, Wv, bv, Wo, bo):
    global _NC, LAST_RESULT
    query, key, value, Wq, bq, Wk, bk, Wv, bv, Wo, bo = (
        np.asarray(a, dtype=np.float32)
        for a in (query, key, value, Wq, bq, Wk, bk, Wv, bv, Wo, bo)
    )
    if _NC is None:
        _NC = _build_nc()

    in_maps = []
    for c in range(NCORES):
        b, g = divmod(c, 4)
        rows = slice(g * DQ, (g + 1) * DQ)
        in_maps.append({
            "xqT": _chunk_major(query[b]),
            "xkT": _chunk_major(key[b]),
            "xvT": _chunk_major(value[b]),
            "wqT": np.ascontiguousarray(Wq[rows].T),
            "wkT": np.ascontiguousarray(Wk[rows].T),
            "wvT": np.ascontiguousarray(Wv[rows].T),
            "woT": np.ascontiguousarray(Wo[:, rows].T),
            "bq": np.ascontiguousarray(bq[rows][None, :]),
            "bk": np.ascontiguousarray(bk[rows][None, :]),
            "bv": np.ascontiguousarray(bv[rows][None, :]),
        })

    res = bass_utils.run_bass_kernel_spmd(
        _NC, in_maps, core_ids=list(range(NCORES)), trace=TRACE
    )
    LAST_RESULT = res

    out = np.zeros((B, S, EMB), np.float32)
    for c in range(NCORES):
        out[c // 4] += res.results[c]["out"]
    out += bo[None, None, :]
    return out



# revision 47
# speedup vs baseline: 1.0324x; 1.0324x over previous
"""Multi-head attention (B=2, S=2048, EMB=1024, 16 heads) on 8 Trainium2 cores.

Sharding: core c -> batch c//4, head-group c%4 (4 heads = 256 projection dims).
Each core computes Q/K projections in transposed layout (head-dim on
partitions), V natural, attention without max subtraction (scores ~ N(0,1) in
fp32), the softmax denominator via a ones-column appended to V (free inside
the ctx matmul M=65), and a row-parallel partial of the output projection.
The host sums the 4 partials per batch and adds the output bias.

v2 design (driven by the TimelineSim cost model, which tracks HW within ~6%):
  - matmul cost = out free-size x cycles/col, independent of K -> ctx matmuls
    use single K=128 chains (16 kt steps) instead of K=64 half-chains,
    halving ctx PE time. Consecutive mms alternate PSUM banks (hi index) so
    LDWEIGHTS pipelines on HW.
  - exp runs on [128, 1024] tiles (both heads of a pair share one 2-bank
    PSUM score tile) halving the per-instruction ACT access overhead.
  - all SBUF/DRAM tensors are bf16 (PSUM stays fp32): same PE rate, half
    the DMA traffic (phase 1 was DMA-bound), half the SBUF footprint.
  - x inputs arrive as one [128, 4096] slab DMA per (tensor, chunk) spread
    over the SP/ACT/DVE queue rings (96 small DMAs -> 12 big ones).
  - softmax 1/den broadcast: one K=2 matmul per head-pair fans both heads'
    reciprocal rows across 128 partitions (sel2 one-hot).
  - normalization is deferred by one head-pair, out-projection by one query
    chunk; both are sliced into the ACT-bound attention loop (one slice per
    kt step) where PE/DVE have slack.
PSUM: spoolA/B 1x[128,1024] each (scores / projections / bcast rotating,
4 banks) + ctx pool 4x[128,512] (ctx chains + out-proj, 4 banks) = 8 banks.
"""

import numpy as np

import concourse.tile as tile
from concourse import bacc, mybir
from concourse import bass_utils

EMB = 1024
S = 2048
B = 2
HPC = 4            # heads per core
DQ = HPC * 64      # 256 projection dims per core
NCORES = 8

F32 = mybir.dt.float32
BF16 = mybir.dt.bfloat16
EXP = mybir.ActivationFunctionType.Exp

KT_E = EMB // 128  # 8 contraction tiles over EMB
NQC = S // 512     # 4 query chunks
NST = S // 128     # 16 sequence tiles

NP_BF16 = mybir.dt.np(BF16)

_NC = None
TRACE = False
LAST_RESULT = None


def _mha(ctx, tc, xqT, xkT, xvT, wqT, wkT, wvT, woT, bq, bk, bv, out, bench_iters=None):
    nc = tc.nc

    cstp = ctx.enter_context(tc.tile_pool(name="const", bufs=1))
    xpool = ctx.enter_context(tc.tile_pool(name="xin", bufs=6))
    epool = ctx.enter_context(tc.tile_pool(name="exp", bufs=4))
    opool = ctx.enter_context(tc.tile_pool(name="osb", bufs=3))
    spoolA = ctx.enter_context(tc.tile_pool(name="scpsA", bufs=1, space="PSUM"))
    spoolB = ctx.enter_context(tc.tile_pool(name="scpsB", bufs=1, space="PSUM"))
    cpool = ctx.enter_context(tc.tile_pool(name="ctxps", bufs=4, space="PSUM"))
    bspool = ctx.enter_context(tc.tile_pool(name="bssb", bufs=2))

    # ---- persistent SBUF tensors ----
    ones_row = cstp.tile([1, 512], BF16)
    # 1/den broadcast operands: the two reciprocal rows live at partitions 0
    # and 64 (engine partition starts must be 0/32/64/96); rows in between
    # stay zero so they contribute nothing to the K=65 broadcast matmul.
    sel2 = cstp.tile([65, 128], BF16)           # row0 -> cols 0:64, row64 -> 64:128
    rdens = [cstp.tile([65, 512], BF16, name=f"rden{i}") for i in range(2)]
    with nc.allow_low_precision(reason="bf16 one-hot constants"):
        nc.vector.memset(ones_row[:], 1.0)
        nc.vector.memset(sel2[:], 0.0)
        nc.vector.memset(sel2[0:1, 0:64], 1.0)
        nc.vector.memset(sel2[64:65, 64:128], 1.0)
        for r in rdens:
            nc.vector.memset(r[:], 0.0)

    wq_sb = cstp.tile([128, KT_E * DQ], BF16)   # [128, 2048]: wq_sb[p, n*256+m] = WqT[n*128+p, m]
    wk_sb = cstp.tile([128, KT_E * DQ], BF16)
    wv_sb = cstp.tile([128, KT_E * DQ], BF16)
    wo_sb = cstp.tile([128, 2 * EMB], BF16)     # wo_sb[p, n*1024+f] = WoT[n*128+p, f]
    # per-partition bias columns: b?c[p, dq] = bias[dq*128+p]
    bqc = cstp.tile([128, 2], F32)
    bkc = cstp.tile([128, 2], F32)
    bv_sb = cstp.tile([1, DQ], BF16)

    # results of phase 1 kept resident
    kT_sb = cstp.tile([128, 2 * S], BF16)       # [dq-block 2][s 2048]
    qT_sb = cstp.tile([128, 2 * S], BF16)
    ctxT_sb = cstp.tile([128, 2 * S], BF16)
    v_sb = cstp.tile([128, NST * (HPC * 65)], BF16)  # per s-tile: 4 heads x (64 V + ones col)
    with nc.allow_low_precision(reason="bf16 ones column"):
        nc.vector.memset(
            v_sb[:].rearrange("p (t h m) -> p t h m", t=NST, h=HPC)[:, :, :, 64:65],
            1.0,
        )

    # ---- weight / bias DMAs; wk leads the SP ring so the first projection
    # starts ASAP. Slow SWDGE (gpsimd) ring only carries non-urgent copies. ----
    nc.sync.dma_start(
        wk_sb[:].rearrange("p (n m) -> p n m", n=KT_E),
        wkT.rearrange("(n p) m -> p n m", p=128),
    )
    nc.scalar.dma_start(
        wv_sb[:].rearrange("p (n m) -> p n m", n=KT_E),
        wvT.rearrange("(n p) m -> p n m", p=128),
    )
    nc.gpsimd.dma_start(bkc[:], bk.rearrange("o (d p) -> p (o d)", p=128))
    nc.gpsimd.dma_start(bqc[:], bq.rearrange("o (d p) -> p (o d)", p=128))
    nc.gpsimd.dma_start(bv_sb[:], bv[:])
    nc.gpsimd.dma_start(
        wq_sb[:].rearrange("p (n m) -> p n m", n=KT_E),
        wqT.rearrange("(n p) m -> p n m", p=128),
    )
    nc.gpsimd.dma_start(
        wo_sb[:].rearrange("p (n m) -> p n m", n=2),
        woT.rearrange("(n p) m -> p n m", p=128),
    )

    def body():
        _body(tc, nc, xqT, xkT, xvT, out, ones_row, sel2, rdens, wq_sb, wk_sb,
              wv_sb, wo_sb, bqc, bkc, bv_sb, kT_sb, qT_sb, ctxT_sb, v_sb,
              xpool, epool, opool, spoolA, spoolB, cpool, bspool)

    if bench_iters:
        hints = (
            mybir.EngineType.PE,
            mybir.EngineType.Activation,
            mybir.EngineType.DVE,
            mybir.EngineType.SP,
            mybir.EngineType.Pool,
        )
        with tc.For_i(0, bench_iters, 1, hint_engines=hints):
            body()
    else:
        body()


def _body(tc, nc, xqT, xkT, xvT, out, ones_row, sel2, rdens, wq_sb, wk_sb,
          wv_sb, wo_sb, bqc, bkc, bv_sb, kT_sb, qT_sb, ctxT_sb, v_sb,
          xpool, epool, opool, spoolA, spoolB, cpool, bspool):
    fin = []  # FIFO of deferred finish slices, popped one per kt step

    def pop():
        if fin:
            fin.pop(0)()

    spools = [spoolA, spoolB]
    sp_state = [0]

    def next_spool(name):
        t = spools[sp_state[0] % 2].tile([128, 1024], F32, tag="sc", name=name)
        sp_state[0] += 1
        return t

    def slab_dma(eng, src, qc, name):
        t = xpool.tile([128, KT_E * 512], BF16, tag="slab", name=name)
        eng.dma_start(t[:], src[qc])
        return t

    def proj_mms(ps, qc, w_sb, xs, kts):
        # two dq chains into the two banks of one [128,1024] psum tile
        for kt in kts:
            for dq in range(2):
                nc.tensor.matmul(
                    ps[:, dq * 512: dq * 512 + 512],
                    w_sb[:, kt * DQ + dq * 128: kt * DQ + dq * 128 + 128],
                    xs[:, kt * 512: kt * 512 + 512],
                    start=(kt == 0), stop=(kt == KT_E - 1),
                )

    def proj_drain(ps, dst_sb, qc, bcol):
        for dq in range(2):
            nc.vector.tensor_scalar_add(
                dst_sb[:, dq * S + qc * 512: dq * S + qc * 512 + 512],
                ps[:, dq * 512: dq * 512 + 512], bcol[:, dq: dq + 1],
            )

    def proj_qk(qc, w_sb, xs, dst_sb, bcol):
        ps = next_spool(f"p_{qc}")
        proj_mms(ps, qc, w_sb, xs, range(KT_E))
        proj_drain(ps, dst_sb, qc, bcol)

    def proj_v(qc, xv):
        for pair in range(2):
            vt = next_spool(f"vps_{qc}_{pair}")
            for kt in range(KT_E):
                for i in range(2):
                    sti = pair * 2 + i
                    nc.tensor.matmul(
                        vt[:, i * 512: i * 512 + DQ],
                        xv[:, kt * 512 + sti * 128: kt * 512 + sti * 128 + 128],
                        wv_sb[:, kt * DQ: kt * DQ + DQ],
                        start=(kt == 0), stop=False,
                    )
            for i in range(2):
                nc.tensor.matmul(
                    vt[:, i * 512: i * 512 + DQ], ones_row[0:1, 0:128], bv_sb[0:1, :],
                    start=False, stop=True,
                )
            for i in range(2):
                st = qc * 4 + pair * 2 + i
                dst = v_sb[:, st * (HPC * 65): (st + 1) * (HPC * 65)]
                nc.vector.tensor_copy(
                    dst.rearrange("p (h m) -> p h m", h=HPC)[:, :, 0:64],
                    vt[:, i * 512: i * 512 + DQ].rearrange("p (h m) -> p h m", h=HPC),
                )

    # ---- finish slices -------------------------------------------------
    # Normalization of (qc, hp): 1/den rows (DVE) -> PE broadcast of each
    # head's 1/den into the spare rows 64:128 of the *other* head's ctx tile
    # (no extra PSUM bank) -> per-head multiply into ctxT_sb.
    def ctx_tail_slice(ctx_mms, prev, hp, cps):
        def go():
            ctx_mms(*prev)
            for hi in range(2):
                with nc.allow_low_precision(reason="bf16 reciprocal of softmax denom"):
                    nc.vector.reciprocal(
                        rdens[hp][64 * hi: 64 * hi + 1, :], cps[hi][64:65, :]
                    )
        return go

    def bcast_slice(qc, hp, st):
        def go():
            bt = next_spool(f"bc_{qc}_{hp}")
            nc.tensor.matmul(bt[:, 0:512], sel2[:], rdens[hp][:], start=True, stop=True)
            bs = bspool.tile([128, 512], BF16, tag="bs", name=f"bs_{qc}_{hp}")
            nc.vector.tensor_copy(bs[:], bt[:, 0:512])
            st.append(bs)
        return go

    def mul_slice(qc, hp, hi, cps, st):
        def go():
            bs = st[0]
            nc.vector.tensor_mul(
                ctxT_sb[64 * hi: 64 * hi + 64, hp * S + qc * 512: hp * S + qc * 512 + 512],
                cps[hi][0:64, :],
                bs[64 * hi: 64 * hi + 64, :],
            )
        return go

    def qproj_mm_slice(ps_box, qc1, xs, step):
        def go():
            if step == 0:
                ps_box.append(
                    [cpool.tile([128, 512], F32, tag="ctx", name=f"qp_{qc1}_{dq}")
                     for dq in range(2)]
            )
            psd = ps_box[0]
            for dq in range(2):
                nc.tensor.matmul(
                    psd[dq][:],
                    wq_sb[:, step * DQ + dq * 128: step * DQ + dq * 128 + 128],
                    xs[:, step * 512: step * 512 + 512],
                    start=(step == 0), stop=(step == KT_E - 1),
                )
        return go

    def qproj_tail_slice(ps_box, qc1):
        def go():
            psd = ps_box[0]
            for dq in range(2):
                nc.vector.tensor_scalar_add(
                    qT_sb[:, dq * S + qc1 * 512: dq * S + qc1 * 512 + 512],
                    psd[dq][:], bqc[:, dq: dq + 1],
                )
        return go

    def queue_qproj(qc1, xs):
        ps_box = []
        for step in range(KT_E):
            fin.append(qproj_mm_slice(ps_box, qc1, xs, step))
        fin.append(qproj_tail_slice(ps_box, qc1))

    def out_mm_slice(qt, st):
        def go():
            ps = [cpool.tile([128, 512], F32, tag="ctx", name=f"ops_{qt}_{fc}")
                  for fc in range(2)]
            for dq in range(2):
                for fc in range(2):
                    nc.tensor.matmul(
                        ps[fc][:],
                        ctxT_sb[:, dq * S + qt * 128: dq * S + qt * 128 + 128],
                        wo_sb[:, dq * EMB + fc * 512: dq * EMB + fc * 512 + 512],
                        start=(dq == 0), stop=(dq == 1),
                    )
            st.append(ps)
        return go

    def out_drain_slice(qt, st):
        def go():
            ps = st.pop(0)
            ot = opool.tile([128, EMB], F32, tag="o", name=f"ot_{qt}")
            nc.vector.tensor_copy(ot[:, 0:512], ps[0][:])
            nc.scalar.activation(ot[:, 512:1024], ps[1][:],
                                 mybir.ActivationFunctionType.Copy)
            nc.gpsimd.dma_start(out[qt * 128:(qt + 1) * 128, :], ot[:])
        return go

    def queue_outproj(qc):
        for qt4 in range(4):
            st = []
            fin.append(out_mm_slice(qc * 4 + qt4, st))
            fin.append(out_drain_slice(qc * 4 + qt4, st))

    # ---- phase 1: K and V projections for every chunk, Q projection for
    # chunk 0 (later chunks' Q is sliced into the attention loop).
    # PE order K0 V0 K1 Q0 V1 K2 V2 K3 V3; DMAs issued in need order across
    # the SP / ACT / Pool queue rings (the DMA engines serialize copies).
    xk0 = xpool.tile([128, KT_E * 512], BF16, tag="slab", name="xk_0")
    nc.scalar.dma_start(xk0[:, 0: KT_E * 256], xkT[0, :, 0: KT_E * 256])
    nc.scalar.dma_start(xk0[:, KT_E * 256:], xkT[0, :, KT_E * 256:])
    xv0 = slab_dma(nc.sync, xvT, 0, "xv_0")
    xk1 = slab_dma(nc.scalar, xkT, 1, "xk_1")
    xq0 = slab_dma(nc.sync, xqT, 0, "xq_0")
    xv1 = slab_dma(nc.scalar, xvT, 1, "xv_1")
    proj_qk(0, wk_sb, xk0, kT_sb, bkc)
    xk2 = slab_dma(nc.sync, xkT, 2, "xk_2")
    proj_v(0, xv0)
    xv2 = slab_dma(nc.scalar, xvT, 2, "xv_2")
    proj_qk(1, wk_sb, xk1, kT_sb, bkc)
    xk3 = slab_dma(nc.sync, xkT, 3, "xk_3")
    proj_qk(0, wq_sb, xq0, qT_sb, bqc)
    xv3 = slab_dma(nc.scalar, xvT, 3, "xv_3")
    proj_v(1, xv1)
    proj_qk(2, wk_sb, xk2, kT_sb, bkc)
    proj_v(2, xv2)
    proj_qk(3, wk_sb, xk3, kT_sb, bkc)
    proj_v(3, xv3)

    # ---- phase 2: attention per query chunk ----
    for qc in range(NQC):
        if qc + 1 < NQC:
            xq_next = slab_dma(nc.sync, xqT, qc + 1, f"xq_{qc + 1}")
        for hp in range(2):
            cps = [cpool.tile([128, 512], F32, tag="ctx", name=f"ctx_{qc}_{hp}_{hi}")
                   for hi in range(2)]

            def ctx_mms(e, kt, hp=hp, cps=cps):
                for hi in range(2):
                    h = hp * 2 + hi
                    vcol = kt * (HPC * 65) + h * 65
                    nc.tensor.matmul(
                        cps[hi][0:65, :], v_sb[:, vcol: vcol + 65],
                        e[:, hi * 512: hi * 512 + 512],
                        start=(kt == 0), stop=(kt == NST - 1),
                    )

            prev = None
            for kt in range(NST):
                sct = next_spool(f"sc_{qc}_{hp}_{kt}")
                for hi in range(2):
                    base = 64 * hi
                    blk = hp * S
                    nc.tensor.matmul(
                        sct[:, hi * 512: hi * 512 + 512],
                        kT_sb[base:base + 64, blk + kt * 128: blk + kt * 128 + 128],
                        qT_sb[base:base + 64, blk + qc * 512: blk + qc * 512 + 512],
                        start=True, stop=True,
                    )
                e = epool.tile([128, 1024], BF16, tag="e", name=f"e_{qc}_{hp}_{kt}")
                nc.scalar.activation(e[:], sct[:], EXP, scale=0.125)
                if prev is not None:
                    ctx_mms(*prev)
                prev = (e, kt)
                pop()
            # last ctx step + normalization are deferred into the next
            # window so the seam never stalls on the final exp
            st = []
            fin.insert(0, ctx_tail_slice(ctx_mms, prev, hp, cps))
            fin.insert(1, bcast_slice(qc, hp, st))
            fin.insert(2, mul_slice(qc, hp, 0, cps, st))
            fin.insert(3, mul_slice(qc, hp, 1, cps, st))
            if hp == 0 and qc + 1 < NQC:
                queue_qproj(qc + 1, xq_next)
        queue_outproj(qc)
    while fin:
        pop()


def _build_nc(bench_iters=None):
    from contextlib import ExitStack

    nc = bacc.Bacc("TRN2", target_bir_lowering=False, debug=False, num_devices=NCORES)
    xqT = nc.dram_tensor("xqT", [NQC, 128, KT_E * 512], BF16, kind="ExternalInput").ap()
    xkT = nc.dram_tensor("xkT", [NQC, 128, KT_E * 512], BF16, kind="ExternalInput").ap()
    xvT = nc.dram_tensor("xvT", [NQC, 128, KT_E * 512], BF16, kind="ExternalInput").ap()
    wqT = nc.dram_tensor("wqT", [EMB, DQ], BF16, kind="ExternalInput").ap()
    wkT = nc.dram_tensor("wkT", [EMB, DQ], BF16, kind="ExternalInput").ap()
    wvT = nc.dram_tensor("wvT", [EMB, DQ], BF16, kind="ExternalInput").ap()
    woT = nc.dram_tensor("woT", [DQ, EMB], BF16, kind="ExternalInput").ap()
    bq = nc.dram_tensor("bq", [1, DQ], F32, kind="ExternalInput").ap()
    bk = nc.dram_tensor("bk", [1, DQ], F32, kind="ExternalInput").ap()
    bv = nc.dram_tensor("bv", [1, DQ], BF16, kind="ExternalInput").ap()
    out = nc.dram_tensor("out", [S, EMB], F32, kind="ExternalOutput").ap()

    with ExitStack() as ctx:
        tc = ctx.enter_context(tile.TileContext(nc))
        _mha(ctx, tc, xqT, xkT, xvT, wqT, wkT, wvT, woT, bq, bk, bv, out,
             bench_iters=bench_iters)
    nc.compile()
    return nc


def _chunk_major(x):
    """[S, EMB] fp32 -> bf16 x.T as [NQC, 128, KT_E*512] (slab per chunk)."""
    xt = np.asarray(x, np.float32).T.astype(NP_BF16)  # [EMB, S]
    return np.ascontiguousarray(
        xt.reshape(KT_E, 128, NQC, 512).transpose(2, 1, 0, 3).reshape(NQC, 128, KT_E * 512)
    )


def make_in_maps(query, key, value, Wq, bq, Wk, bk, Wv, bv, Wo, bo):
    in_maps = []
    for c in range(NCORES):
        b, g = divmod(c, 4)
        rows = slice(g * DQ, (g + 1) * DQ)
        in_maps.append({
            "xqT": _chunk_major(query[b]),
            "xkT": _chunk_major(key[b]),
            "xvT": _chunk_major(value[b]),
            "wqT": np.ascontiguousarray(np.asarray(Wq[rows].T, np.float32).astype(NP_BF16)),
            "wkT": np.ascontiguousarray(np.asarray(Wk[rows].T, np.float32).astype(NP_BF16)),
            "wvT": np.ascontiguousarray(np.asarray(Wv[rows].T, np.float32).astype(NP_BF16)),
            "woT": np.ascontiguousarray(np.asarray(Wo[:, rows].T, np.float32).astype(NP_BF16)),
            "bq": np.ascontiguousarray(np.asarray(bq[rows], np.float32)[None, :]),
            "bk": np.ascontiguousarray(np.asarray(bk[rows], np.float32)[None, :]),
            "bv": np.ascontiguousarray(np.asarray(bv[rows], np.float32).astype(NP_BF16)[None, :]),
        })
    return in_maps


def kernel(query, key, value, Wq, bq, Wk, bk, Wv, bv, Wo, bo):
    global _NC, LAST_RESULT
    if _NC is None:
        _NC = _build_nc()

    in_maps = make_in_maps(query, key, value, Wq, bq, Wk, bk, Wv, bv, Wo, bo)
    res = bass_utils.run_bass_kernel_spmd(
        _NC, in_maps, core_ids=list(range(NCORES)), trace=TRACE
    )
    LAST_RESULT = res

    out = np.zeros((B, S, EMB), np.float32)
    for c in range(NCORES):
        out[c // 4] += res.results[c]["out"]
    out += np.asarray(bo, np.float32)[None, None, :]
    return out


# revision 55
# speedup vs baseline: 1.0498x; 1.0168x over previous
"""Multi-head attention (B=2, S=2048, EMB=1024, 16 heads) on 8 Trainium2 cores.

Sharding: core c -> batch c//4, head-group c%4 (4 heads = 256 projection dims).
Each core computes Q/K projections in transposed layout (head-dim on
partitions), V natural, attention without max subtraction (scores ~ N(0,1) in
fp32), the softmax denominator via a ones-column appended to V (free inside
the ctx matmul M=65), and a row-parallel partial of the output projection.
The host sums the 4 partials per batch and adds the output bias.

v2 design (driven by the TimelineSim cost model, which tracks HW within ~6%):
  - matmul cost = out free-size x cycles/col, independent of K -> ctx matmuls
    use single K=128 chains (16 kt steps) instead of K=64 half-chains,
    halving ctx PE time. Consecutive mms alternate PSUM banks (hi index) so
    LDWEIGHTS pipelines on HW.
  - exp runs on [128, 1024] tiles (both heads of a pair share one 2-bank
    PSUM score tile) halving the per-instruction ACT access overhead.
  - all SBUF/DRAM tensors are bf16 (PSUM stays fp32): same PE rate, half
    the DMA traffic (phase 1 was DMA-bound), half the SBUF footprint.
  - x inputs arrive as one [128, 4096] slab DMA per (tensor, chunk) spread
    over the SP/ACT/DVE queue rings (96 small DMAs -> 12 big ones).
  - softmax 1/den broadcast: one K=2 matmul per head-pair fans both heads'
    reciprocal rows across 128 partitions (sel2 one-hot).
  - normalization is deferred by one head-pair, out-projection by one query
    chunk; both are sliced into the ACT-bound attention loop (one slice per
    kt step) where PE/DVE have slack.
PSUM: spoolA/B 1x[128,1024] each (scores / projections / bcast rotating,
4 banks) + ctx pool 4x[128,512] (ctx chains + out-proj, 4 banks) = 8 banks.
"""

import numpy as np

import concourse.tile as tile
from concourse import bacc, mybir
from concourse import bass_utils

EMB = 1024
S = 2048
B = 2
HPC = 4            # heads per core
DQ = HPC * 64      # 256 projection dims per core
NCORES = 8

F32 = mybir.dt.float32
BF16 = mybir.dt.bfloat16
EXP = mybir.ActivationFunctionType.Exp

KT_E = EMB // 128  # 8 contraction tiles over EMB
NQC = S // 512     # 4 query chunks
NST = S // 128     # 16 sequence tiles

NP_BF16 = mybir.dt.np(BF16)

_NC = None
TRACE = False
LAST_RESULT = None
STAGE = "full"   # "full" | "nofin" (skip norm/outproj/qproj) | "noodma" (skip out DMA) | "noxdma" (reuse one slab)


def _mha(ctx, tc, xqT, xkT, xvT, wqT, wkT, wvT, woT, bq, bk, bv, out, bench_iters=None):
    nc = tc.nc

    cstp = ctx.enter_context(tc.tile_pool(name="const", bufs=1))
    xpool = ctx.enter_context(tc.tile_pool(name="xin", bufs=6))
    epool = ctx.enter_context(tc.tile_pool(name="exp", bufs=4))
    opool = ctx.enter_context(tc.tile_pool(name="osb", bufs=3))
    spoolA = ctx.enter_context(tc.tile_pool(name="scpsA", bufs=1, space="PSUM"))
    spoolB = ctx.enter_context(tc.tile_pool(name="scpsB", bufs=1, space="PSUM"))
    cpool = ctx.enter_context(tc.tile_pool(name="ctxps", bufs=4, space="PSUM"))
    upool = ctx.enter_context(tc.tile_pool(name="unorm", bufs=4))

    # ---- persistent SBUF tensors ----
    ones_row = cstp.tile([1, 512], BF16)
    # 1/den broadcast operands: the two reciprocal rows live at partitions 0
    # and 64 (engine partition starts must be 0/32/64/96); rows in between
    # stay zero so they contribute nothing to the K=65 broadcast matmul.
    sel2 = cstp.tile([65, 128], BF16)           # row0 -> cols 0:64, row64 -> 64:128
    rdens = [cstp.tile([65, 512], BF16, name=f"rden{i}") for i in range(2)]
    with nc.allow_low_precision(reason="bf16 one-hot constants"):
        nc.vector.memset(ones_row[:], 1.0)
        nc.vector.memset(sel2[:], 0.0)
        nc.vector.memset(sel2[0:1, 0:64], 1.0)
        nc.vector.memset(sel2[64:65, 64:128], 1.0)
        for r in rdens:
            nc.vector.memset(r[:], 0.0)

    wq_sb = cstp.tile([128, KT_E * DQ], BF16)   # [128, 2048]: wq_sb[p, n*256+m] = WqT[n*128+p, m]
    wk_sb = cstp.tile([128, KT_E * DQ], BF16)
    wv_sb = cstp.tile([128, KT_E * DQ], BF16)
    wo_sb = cstp.tile([128, 2 * EMB], BF16)     # wo_sb[p, n*1024+f] = WoT[n*128+p, f]
    # per-partition bias columns: b?c[p, dq] = bias[dq*128+p]
    bqc = cstp.tile([128, 2], F32)
    bkc = cstp.tile([128, 2], F32)
    bv_sb = cstp.tile([1, DQ], BF16)

    preslabs = {}
    if STAGE == "noxdma":
        for nm, src, qc in ([(f"xk_{i}", xkT, i) for i in range(NQC)]
                            + [(f"xv_{i}", xvT, i) for i in range(NQC)]
                            + [(f"xq_{i}", xqT, i) for i in range(NQC)]):
            t = cstp.tile([128, KT_E * 512], BF16, name=f"pre_{nm}")
            nc.sync.dma_start(t[:], src[qc])
            preslabs[nm] = t

    # results of phase 1 kept resident
    kT_sb = cstp.tile([128, 2 * S], BF16)       # [dq-block 2][s 2048]
    qT_sb = cstp.tile([128, 2 * S], BF16)
    ctxT_sb = cstp.tile([128, 2 * S], BF16)
    v_sb = cstp.tile([128, NST * (HPC * 65)], BF16)  # per s-tile: 4 heads x (64 V + ones col)
    with nc.allow_low_precision(reason="bf16 ones column"):
        nc.vector.memset(
            v_sb[:].rearrange("p (t h m) -> p t h m", t=NST, h=HPC)[:, :, :, 64:65],
            1.0,
        )

    # ---- weight / bias DMAs; wk leads the SP ring so the first projection
    # starts ASAP. Slow SWDGE (gpsimd) ring only carries non-urgent copies. ----
    nc.sync.dma_start(
        wk_sb[:].rearrange("p (n m) -> p n m", n=KT_E),
        wkT.rearrange("(n p) m -> p n m", p=128),
    )
    nc.scalar.dma_start(
        wv_sb[:].rearrange("p (n m) -> p n m", n=KT_E),
        wvT.rearrange("(n p) m -> p n m", p=128),
    )
    nc.gpsimd.dma_start(bkc[:], bk.rearrange("o (d p) -> p (o d)", p=128))
    nc.gpsimd.dma_start(bqc[:], bq.rearrange("o (d p) -> p (o d)", p=128))
    nc.gpsimd.dma_start(bv_sb[:], bv[:])
    nc.gpsimd.dma_start(
        wq_sb[:].rearrange("p (n m) -> p n m", n=KT_E),
        wqT.rearrange("(n p) m -> p n m", p=128),
    )
    nc.gpsimd.dma_start(
        wo_sb[:].rearrange("p (n m) -> p n m", n=2),
        woT.rearrange("(n p) m -> p n m", p=128),
    )

    def body():
        _body(tc, nc, xqT, xkT, xvT, out, ones_row, sel2, rdens, wq_sb, wk_sb,
              wv_sb, wo_sb, bqc, bkc, bv_sb, kT_sb, qT_sb, ctxT_sb, v_sb,
              xpool, epool, opool, spoolA, spoolB, cpool, upool, preslabs)

    if bench_iters:
        hints = (
            mybir.EngineType.PE,
            mybir.EngineType.Activation,
            mybir.EngineType.DVE,
            mybir.EngineType.SP,
            mybir.EngineType.Pool,
        )
        with tc.For_i(0, bench_iters, 1, hint_engines=hints):
            body()
    else:
        body()


def _body(tc, nc, xqT, xkT, xvT, out, ones_row, sel2, rdens, wq_sb, wk_sb,
          wv_sb, wo_sb, bqc, bkc, bv_sb, kT_sb, qT_sb, ctxT_sb, v_sb,
          xpool, epool, opool, spoolA, spoolB, cpool, upool, preslabs):
    fin = []  # FIFO of deferred finish slices, popped one per kt step

    def pop():
        if fin:
            fin.pop(0)()

    def queue(idx, sl):
        if STAGE not in ("nofin", "nonorm"):
            fin.insert(idx, sl) if idx is not None else fin.append(sl)

    spools = [spoolA, spoolB]
    sp_state = [0]

    def next_spool(name):
        t = spools[sp_state[0] % 2].tile([128, 1024], F32, tag="sc", name=name)
        sp_state[0] += 1
        return t

    def slab_dma(eng, src, qc, name):
        if STAGE == "noxdma":
            return preslabs[name]
        t = xpool.tile([128, KT_E * 512], BF16, tag="slab", name=name)
        eng.dma_start(t[:], src[qc])
        return t

    def proj_mms(ps, qc, w_sb, xs, kts):
        # two dq chains into the two banks of one [128,1024] psum tile
        for kt in kts:
            for dq in range(2):
                nc.tensor.matmul(
                    ps[:, dq * 512: dq * 512 + 512],
                    w_sb[:, kt * DQ + dq * 128: kt * DQ + dq * 128 + 128],
                    xs[:, kt * 512: kt * 512 + 512],
                    start=(kt == 0), stop=(kt == KT_E - 1),
                )

    def proj_drain(ps, dst_sb, qc, bcol):
        for dq in range(2):
            nc.vector.tensor_scalar_add(
                dst_sb[:, dq * S + qc * 512: dq * S + qc * 512 + 512],
                ps[:, dq * 512: dq * 512 + 512], bcol[:, dq: dq + 1],
            )

    def proj_qk(qc, w_sb, xs, dst_sb, bcol):
        ps = next_spool(f"p_{qc}")
        proj_mms(ps, qc, w_sb, xs, range(KT_E))
        proj_drain(ps, dst_sb, qc, bcol)

    def proj_v(qc, xv):
        for pair in range(2):
            vt = next_spool(f"vps_{qc}_{pair}")
            for kt in range(KT_E):
                for i in range(2):
                    sti = pair * 2 + i
                    nc.tensor.matmul(
                        vt[:, i * 512: i * 512 + DQ],
                        xv[:, kt * 512 + sti * 128: kt * 512 + sti * 128 + 128],
                        wv_sb[:, kt * DQ: kt * DQ + DQ],
                        start=(kt == 0), stop=False,
                    )
            for i in range(2):
                nc.tensor.matmul(
                    vt[:, i * 512: i * 512 + DQ], ones_row[0:1, 0:128], bv_sb[0:1, :],
                    start=False, stop=True,
                )
            for i in range(2):
                st = qc * 4 + pair * 2 + i
                dst = v_sb[:, st * (HPC * 65): (st + 1) * (HPC * 65)]
                nc.vector.tensor_copy(
                    dst.rearrange("p (h m) -> p h m", h=HPC)[:, :, 0:64],
                    vt[:, i * 512: i * 512 + DQ].rearrange("p (h m) -> p h m", h=HPC),
                )

    # ---- finish slices -------------------------------------------------
    # Normalization of (qc, hp): 1/den rows (DVE) -> PE broadcast of each
    # head's 1/den into the spare rows 64:128 of the *other* head's ctx tile
    # (no extra PSUM bank) -> per-head multiply into ctxT_sb.
    def ctx_tail_slice(ctx_mms, prev, hp, cps):
        def go():
            ctx_mms(*prev)
            for hi in range(2):
                with nc.allow_low_precision(reason="bf16 reciprocal of softmax denom"):
                    nc.vector.reciprocal(
                        rdens[hp][64 * hi: 64 * hi + 1, :], cps[hi][64:65, :]
                    )
        return go

    def ucopy_slice(qc, hp, cps, st):
        def go():
            for hi in range(2):
                u = upool.tile([64, 512], BF16, tag="u", name=f"u_{qc}_{hp}_{hi}")
                nc.vector.tensor_copy(u[:], cps[hi][0:64, :])
                st.append(u)
        return go

    def bcast_slice(qc, hp, st):
        def go():
            bt = cpool.tile([128, 512], F32, tag="ctx", name=f"bc_{qc}_{hp}")
            nc.tensor.matmul(bt[:], sel2[:], rdens[hp][:], start=True, stop=True)
            st.append(bt)
        return go

    def mul_slice(qc, hp, hi, st):
        def go():
            bt = st[2]
            nc.vector.tensor_mul(
                ctxT_sb[64 * hi: 64 * hi + 64, hp * S + qc * 512: hp * S + qc * 512 + 512],
                st[hi][:],
                bt[64 * hi: 64 * hi + 64, :],
            )
        return go

    def qproj_mm_slice(ps_box, qc1, xs, step):
        def go():
            if step == 0:
                ps_box.append(
                    [cpool.tile([128, 512], F32, tag="ctx", name=f"qp_{qc1}_{dq}")
                     for dq in range(2)]
            )
            psd = ps_box[0]
            for dq in range(2):
                nc.tensor.matmul(
                    psd[dq][:],
                    wq_sb[:, step * DQ + dq * 128: step * DQ + dq * 128 + 128],
                    xs[:, step * 512: step * 512 + 512],
                    start=(step == 0), stop=(step == KT_E - 1),
                )
        return go

    def qproj_tail_slice(ps_box, qc1):
        def go():
            psd = ps_box[0]
            for dq in range(2):
                nc.vector.tensor_scalar_add(
                    qT_sb[:, dq * S + qc1 * 512: dq * S + qc1 * 512 + 512],
                    psd[dq][:], bqc[:, dq: dq + 1],
                )
        return go

    def queue_qproj(qc1, xs):
        ps_box = []
        for step in range(KT_E):
            fin.append(qproj_mm_slice(ps_box, qc1, xs, step))
        fin.append(qproj_tail_slice(ps_box, qc1))

    def out_mm_slice(qt, st):
        def go():
            ps = [cpool.tile([128, 512], F32, tag="ctx", name=f"ops_{qt}_{fc}")
                  for fc in range(2)]
            for dq in range(2):
                for fc in range(2):
                    nc.tensor.matmul(
                        ps[fc][:],
                        ctxT_sb[:, dq * S + qt * 128: dq * S + qt * 128 + 128],
                        wo_sb[:, dq * EMB + fc * 512: dq * EMB + fc * 512 + 512],
                        start=(dq == 0), stop=(dq == 1),
                    )
            st.append(ps)
        return go

    def out_drain_slice(qt, st):
        def go():
            ps = st.pop(0)
            ot = opool.tile([128, EMB], F32, tag="o", name=f"ot_{qt}")
            for fc in range(2):
                nc.vector.tensor_copy(ot[:, fc * 512: fc * 512 + 512], ps[fc][:])
            if STAGE != "noodma":
                nc.gpsimd.dma_start(out[qt * 128:(qt + 1) * 128, :], ot[:])
        return go

    def queue_outproj(qc):
        for qt4 in range(4):
            st = []
            fin.append(out_mm_slice(qc * 4 + qt4, st))
            fin.append(out_drain_slice(qc * 4 + qt4, st))

    # ---- phase 1: K and V projections for every chunk, Q projection for
    # chunk 0 (later chunks' Q is sliced into the attention loop).
    # PE order K0 V0 K1 Q0 V1 K2 V2 K3 V3; DMAs issued in need order across
    # the SP / ACT / Pool queue rings (the DMA engines serialize copies).
    if STAGE == "noxdma":
        xk0 = preslabs["xk_0"]
    else:
        xk0 = xpool.tile([128, KT_E * 512], BF16, tag="slab", name="xk_0")
        nc.scalar.dma_start(xk0[:, 0: KT_E * 256], xkT[0, :, 0: KT_E * 256])
        nc.scalar.dma_start(xk0[:, KT_E * 256:], xkT[0, :, KT_E * 256:])
    xv0 = slab_dma(nc.sync, xvT, 0, "xv_0")
    xk1 = slab_dma(nc.scalar, xkT, 1, "xk_1")
    xq0 = slab_dma(nc.sync, xqT, 0, "xq_0")
    xv1 = slab_dma(nc.scalar, xvT, 1, "xv_1")
    proj_qk(0, wk_sb, xk0, kT_sb, bkc)
    xk2 = slab_dma(nc.sync, xkT, 2, "xk_2")
    proj_v(0, xv0)
    xv2 = slab_dma(nc.scalar, xvT, 2, "xv_2")
    proj_qk(1, wk_sb, xk1, kT_sb, bkc)
    xk3 = slab_dma(nc.sync, xkT, 3, "xk_3")
    proj_qk(0, wq_sb, xq0, qT_sb, bqc)
    xv3 = slab_dma(nc.scalar, xvT, 3, "xv_3")
    proj_v(1, xv1)
    proj_qk(2, wk_sb, xk2, kT_sb, bkc)
    proj_v(2, xv2)
    proj_qk(3, wk_sb, xk3, kT_sb, bkc)
    proj_v(3, xv3)

    # ---- phase 2: attention per query chunk ----
    if STAGE == "ph1only":
        return
    for qc in range(NQC):
        qcq = 0 if STAGE == "nofin" else qc
        if qc + 1 < NQC and STAGE != "nofin":
            xq_next = slab_dma(nc.sync, xqT, qc + 1, f"xq_{qc + 1}")
        for hp in range(2):
            cps = [cpool.tile([128, 512], F32, tag="ctx", name=f"ctx_{qc}_{hp}_{hi}")
                   for hi in range(2)]

            def ctx_mms(e, kt, hp=hp, cps=cps):
                for hi in range(2):
                    h = hp * 2 + hi
                    vcol = kt * (HPC * 65) + h * 65
                    nc.tensor.matmul(
                        cps[hi][0:65, :], v_sb[:, vcol: vcol + 65],
                        e[:, hi * 512: hi * 512 + 512],
                        start=(kt == 0), stop=(kt == NST - 1),
                    )

            prev = None
            for kt in range(NST):
                sct = next_spool(f"sc_{qc}_{hp}_{kt}")
                for hi in range(2):
                    base = 64 * hi
                    blk = hp * S
                    nc.tensor.matmul(
                        sct[:, hi * 512: hi * 512 + 512],
                        kT_sb[base:base + 64, blk + kt * 128: blk + kt * 128 + 128],
                        qT_sb[base:base + 64, blk + qcq * 512: blk + qcq * 512 + 512],
                        start=True, stop=True,
                    )
                e = epool.tile([128, 1024], BF16, tag="e", name=f"e_{qc}_{hp}_{kt}")
                nc.scalar.activation(e[:], sct[:], EXP, scale=0.125)
                if prev is not None:
                    ctx_mms(*prev)
                prev = (e, kt)
                pop()
            # last ctx step + normalization are deferred into the next
            # window so the seam never stalls on the final exp
            st = []
            fin.insert(0, ctx_tail_slice(ctx_mms, prev, hp, cps))
            queue(1, ucopy_slice(qc, hp, cps, st))
            queue(2, bcast_slice(qc, hp, st))
            queue(3, mul_slice(qc, hp, 0, st))
            queue(4, mul_slice(qc, hp, 1, st))
            queue(5, lambda: None)
            if hp == 0 and qc + 1 < NQC and STAGE not in ("nofin", "noqp"):
                if STAGE == "qpseam":
                    ps = next_spool(f"qp_{qc + 1}")
                    for dq in range(2):
                        for step in range(KT_E):
                            nc.tensor.matmul(
                                ps[:, dq * 512: dq * 512 + 512],
                                wq_sb[:, step * DQ + dq * 128: step * DQ + dq * 128 + 128],
                                xq_next[:, step * 512: step * 512 + 512],
                                start=(step == 0), stop=(step == KT_E - 1),
                            )
                    proj_drain(ps, qT_sb, qc + 1, bqc)
                else:
                    queue_qproj(qc + 1, xq_next)
        if STAGE not in ("nofin", "noout"):
            queue_outproj(qc)
    while fin:
        pop()


def _build_nc(bench_iters=None):
    from contextlib import ExitStack

    nc = bacc.Bacc("TRN2", target_bir_lowering=False, debug=False, num_devices=NCORES)
    xqT = nc.dram_tensor("xqT", [NQC, 128, KT_E * 512], BF16, kind="ExternalInput").ap()
    xkT = nc.dram_tensor("xkT", [NQC, 128, KT_E * 512], BF16, kind="ExternalInput").ap()
    xvT = nc.dram_tensor("xvT", [NQC, 128, KT_E * 512], BF16, kind="ExternalInput").ap()
    wqT = nc.dram_tensor("wqT", [EMB, DQ], BF16, kind="ExternalInput").ap()
    wkT = nc.dram_tensor("wkT", [EMB, DQ], BF16, kind="ExternalInput").ap()
    wvT = nc.dram_tensor("wvT", [EMB, DQ], BF16, kind="ExternalInput").ap()
    woT = nc.dram_tensor("woT", [DQ, EMB], BF16, kind="ExternalInput").ap()
    bq = nc.dram_tensor("bq", [1, DQ], F32, kind="ExternalInput").ap()
    bk = nc.dram_tensor("bk", [1, DQ], F32, kind="ExternalInput").ap()
    bv = nc.dram_tensor("bv", [1, DQ], BF16, kind="ExternalInput").ap()
    out = nc.dram_tensor("out", [S, EMB], F32, kind="ExternalOutput").ap()

    with ExitStack() as ctx:
        tc = ctx.enter_context(tile.TileContext(nc))
        _mha(ctx, tc, xqT, xkT, xvT, wqT, wkT, wvT, woT, bq, bk, bv, out,
             bench_iters=bench_iters)
    nc.compile()
    return nc


def _chunk_major(x):
    """[S, EMB] fp32 -> bf16 x.T as [NQC, 128, KT_E*512] (slab per chunk)."""
    xt = np.asarray(x, np.float32).T.astype(NP_BF16)  # [EMB, S]
    return np.ascontiguousarray(
        xt.reshape(KT_E, 128, NQC, 512).transpose(2, 1, 0, 3).reshape(NQC, 128, KT_E * 512)
    )


def make_in_maps(query, key, value, Wq, bq, Wk, bk, Wv, bv, Wo, bo):
    in_maps = []
    for c in range(NCORES):
        b, g = divmod(c, 4)
        rows = slice(g * DQ, (g + 1) * DQ)
        in_maps.append({
            "xqT": _chunk_major(query[b]),
            "xkT": _chunk_major(key[b]),
            "xvT": _chunk_major(value[b]),
            "wqT": np.ascontiguousarray(np.asarray(Wq[rows].T, np.float32).astype(NP_BF16)),
            "wkT": np.ascontiguousarray(np.asarray(Wk[rows].T, np.float32).astype(NP_BF16)),
            "wvT": np.ascontiguousarray(np.asarray(Wv[rows].T, np.float32).astype(NP_BF16)),
            "woT": np.ascontiguousarray(np.asarray(Wo[:, rows].T, np.float32).astype(NP_BF16)),
            "bq": np.ascontiguousarray(np.asarray(bq[rows], np.float32)[None, :]),
            "bk": np.ascontiguousarray(np.asarray(bk[rows], np.float32)[None, :]),
            "bv": np.ascontiguousarray(np.asarray(bv[rows], np.float32).astype(NP_BF16)[None, :]),
        })
    return in_maps


def kernel(query, key, value, Wq, bq, Wk, bk, Wv, bv, Wo, bo):
    global _NC, LAST_RESULT
    if _NC is None:
        _NC = _build_nc()

    in_maps = make_in_maps(query, key, value, Wq, bq, Wk, bk, Wv, bv, Wo, bo)
    res = bass_utils.run_bass_kernel_spmd(
        _NC, in_maps, core_ids=list(range(NCORES)), trace=TRACE
    )
    LAST_RESULT = res

    out = np.zeros((B, S, EMB), np.float32)
    for c in range(NCORES):
        out[c // 4] += res.results[c]["out"]
    out += np.asarray(bo, np.float32)[None, None, :]
    return out
